# revision 27
# baseline (speedup 1.0000x reference)
"""Trainium2 Bass kernel for nn_DiscoStem (B=64, 8 layers of halving BiLSTM).

Sharding: pure data parallel over batch (B=64 -> 8 cores x 8 rows each), per
the sharding hint; all parameters replicated.  Each core runs the whole layer
pipeline for its batch shard in ONE SPMD Bass program:

  - x is kept transposed per layer in DRAM: xT_l[4][128][BC*n] (d-major, token
    column = b*n + t), so every matmul contraction is a plain [128, N] tile
    and all token gathers / conv shifts are free-dim strided APs.
  - LSTM scan (the latency-critical part): gate tiles [128=H, (gate,dir,b)],
    fwd+bwd chains share each step's instructions.  Per step: 2 xp-inject
    matmuls (identity trick, fp16) + 8 whh gate matmuls (fp16, FWL) + one
    fused sigmoid over (f,i,o), tanh(g), then 3 packed DVE ops for the
    c/h update.  xp is precomputed slot-major per layer (one big fp32r GEMM).
  - split/merge gathers: host computes the exact reference indices; affine
    patterns (all reference-style inputs: clean pairing, or all-ones condense)
    are baked as strided APs.  Non-affine patterns are not supported.
  - combine gate + blend: sigmoid on ACT; x' = lhs + (g-1)*jnt*(lhs-rhs) via
    scalar_tensor_tensor + tensor_tensor ops.

Host side: prepares transposed/permuted weights (gate order f,i,o,g), runs the
SPMD program on 8 cores via run_bass_kernel_spmd, then assembles the outputs
(rd = rd_f + rd_b + b_ori with bwd slots reversed; embeddings un-transposed;
existence = ones).
"""

import functools

import numpy as np

# ---------------------------------------------------------------- constants
B, SEG, D, OD, H = 64, 256, 512, 256, 128
SEGS = [256, 128, 64, 32, 16, 8, 4, 2]
N_ORI = sum(SEGS)                      # 510
N_JNT = sum(s - 1 for s in SEGS[:-1])  # 501
NC = 8                                 # cores
BC = B // NC                           # batch rows per core (8)
GSRC = [1, 0, 3, 2]                    # target gate k (f,i,o,g) -> pytorch block

OFF_ORI = np.concatenate([[0], np.cumsum(SEGS)]).astype(np.int64)
OFF_JNT = np.concatenate([[0], np.cumsum([s - 1 for s in SEGS[:-1]])]).astype(np.int64)
OFF_X = np.concatenate([[0], np.cumsum([s // 2 for s in SEGS[:-1]])]).astype(np.int64)
S0 = BC * SEG                          # layer-0 scan slots per core
S1 = BC * sum(SEGS[1:])                # layer>=1 scan slots per core (2032)


# ------------------------------------------------------------- host helpers
def _split_merge_idx(right, joint, exist):
    """Exact numpy replica of reference._split_merge index math."""
    bsz, n = right.shape
    n2 = n // 2
    agree = right[:, :-1] & ~right[:, 1:]
    pj = agree & joint
    rhs_is = np.concatenate([np.zeros((bsz, 1), bool), pj], axis=1)
    node_start = exist & ~rhs_is
    node_id = np.cumsum(node_start.astype(np.int64), axis=1) - 1
    pos = np.broadcast_to(np.arange(n, dtype=np.int64), (bsz, n))
    lhs_idx = np.zeros((bsz, n2), np.int64)
    rhs_idx = np.zeros((bsz, n2), np.int64)
    jnt = np.zeros((bsz, n2), bool)
    tgt_l = np.where(node_start, node_id, n2)
    lhs_pj = np.concatenate([pj, np.zeros((bsz, 1), bool)], axis=1)
    tgt_r = np.where(lhs_pj, node_id, n2)
    for b in range(bsz):
        ok = tgt_l[b] < n2
        lhs_idx[b, tgt_l[b][ok]] = pos[b, ok]
        ok = tgt_r[b] < n2
        rhs_idx[b, tgt_r[b][ok]] = pos[b, ok] + 1
        jnt[b, tgt_r[b][ok]] = True
    return lhs_idx, rhs_idx, jnt


def _affine_of(idx):
    """If idx[b, i] == s*i + o for all b, return (s, o); else None."""
    bsz, n2 = idx.shape
    if n2 == 1:
        return (1, int(idx[0, 0])) if (idx == idx[0, 0]).all() else None
    s = int(idx[0, 1] - idx[0, 0])
    o = int(idx[0, 0])
    if s < 0:
        return None
    ref = s * np.arange(n2, dtype=np.int64)[None, :] + o
    return (s, o) if (idx == ref).all() else None


# --------------------------------------------------------------- the kernel
def kernel(**inputs):
    unit_emb = np.asarray(inputs["unit_emb"], np.float32)
    existence = np.asarray(inputs["existence"], bool)
    sup_right = np.asarray(inputs["supervised_right"], bool)
    sup_joint = np.asarray(inputs["supervised_joint"], bool)
    h0 = np.asarray(inputs["h0"], np.float32)
    c0 = np.asarray(inputs["c0"], np.float32)
    b_ori = np.asarray(inputs["b_ori"], np.float32)
    b_jnt = np.asarray(inputs["b_jnt"], np.float32)

    # ---- host: per-layer gather indices (exact) + affine detection
    ex = existence
    ori_off = jnt_off = 0
    affines = []
    jmasks = []
    n = SEG
    for l in range(7):
        right = sup_right[:, ori_off:ori_off + n]; ori_off += n
        joint = sup_joint[:, jnt_off:jnt_off + n - 1]; jnt_off += n - 1
        lhs_idx, rhs_idx, jnt = _split_merge_idx(right, joint, ex)
        la, ra = _affine_of(lhs_idx), _affine_of(rhs_idx)
        if la is None or ra is None:
            raise NotImplementedError(
                "non-affine split/merge index pattern is not supported")
        affines.append((*la, *ra))
        jmasks.append(jnt.astype(np.float32))
        ex = np.ones((B, n // 2), bool)
        n //= 2

    # ---- host: weights in device layouts (gate order f,i,o,g)
    def perm_gates(w):
        return np.concatenate([w[g * H:(g + 1) * H] for g in GSRC], axis=0)

    wih = {0: perm_gates(np.asarray(inputs["wih_f"], np.float32)),
           1: perm_gates(np.asarray(inputs["wih_b"], np.float32))}
    whh = {0: perm_gates(np.asarray(inputs["whh_f"], np.float32)),
           1: perm_gates(np.asarray(inputs["whh_b"], np.float32))}
    bih = {0: perm_gates(np.asarray(inputs["b_f"], np.float32)[:, None])[:, 0],
           1: perm_gates(np.asarray(inputs["b_b"], np.float32)[:, None])[:, 0]}
    # tanh(g) = 2*sigmoid(2g) - 1: fold the 2x into the g-gate block so the
    # scan needs a single fused sigmoid over all four gates
    for d in (0, 1):
        wih[d] = wih[d].copy(); whh[d] = whh[d].copy(); bih[d] = bih[d].copy()
        wih[d][3 * H:4 * H] *= 2.0
        whh[d][3 * H:4 * H] *= 2.0
        bih[d][3 * H:4 * H] *= 2.0
    w_ori = np.asarray(inputs["w_ori"], np.float32)
    w_cnv = np.asarray(inputs["w_cnv"], np.float32)
    b_cnv = np.asarray(inputs["b_cnv"], np.float32)
    w_jnt = np.asarray(inputs["w_jnt"], np.float32)
    w_cmb = np.asarray(inputs["w_cmb"], np.float32)
    b_cmb = np.asarray(inputs["b_cmb"], np.float32)

    wihT = np.stack([
        np.stack([np.stack([wih[d][k * H:(k + 1) * H, c * H:(c + 1) * H].T
                            for k in range(4)]) for c in range(4)])
        for d in (0, 1)]).astype(np.float16)                # [2,4,4,128,128]
    whhT16 = np.stack([whh[d].T for d in (0, 1)]).astype(np.float16)  # [2,128,512]
    biht = np.stack([bih[d].reshape(4, H).T for d in (0, 1)], axis=0)
    biht = np.ascontiguousarray(biht.transpose(1, 0, 2)).reshape(H, 8)  # [128,(d,k)]
    wcmT = np.stack([np.stack([w_cmb[c * H:(c + 1) * H, m * H:(m + 1) * H]
                               for m in range(4)]) for c in range(8)]).astype(np.float16)
    bcmt = np.ascontiguousarray(b_cmb.reshape(4, H).T)      # [128, 4]
    wcv = np.concatenate([w_cnv[0], w_cnv[1]], axis=0)      # [1024, 256]
    wcvT = np.stack([np.stack([wcv[c * H:(c + 1) * H, m * H:(m + 1) * H]
                               for m in range(2)]) for c in range(8)]).astype(np.float16)
    bcvt = np.ascontiguousarray(b_cnv.reshape(2, H).T)      # [128, 2]
    wjT = np.ascontiguousarray(w_jnt[:, 0].reshape(2, H).T).astype(np.float16)
    worT16 = np.concatenate([w_ori[:H], w_ori[H:]], axis=1).astype(np.float16)
    h0i = np.tanh(h0)
    h0t = np.concatenate([np.broadcast_to(h0i[d, 0][:, None], (H, BC))
                          for d in (0, 1)], axis=1).astype(np.float16)
    c0t = np.concatenate([np.broadcast_to(c0[d, 0][:, None], (H, BC))
                          for d in (0, 1)], axis=1).astype(np.float32)
    ident16 = np.eye(H, dtype=np.float16)

    # pack all constant tensors into two [H, X] arrays (one DMA each)
    def pack(blocks):
        cols = np.concatenate(blocks, axis=1)
        return np.ascontiguousarray(cols)

    p16_blocks = []
    p16_off = {}

    def add16(name, arr):
        p16_off[name] = sum(b.shape[1] for b in p16_blocks)
        p16_blocks.append(arr.astype(np.float16))

    for d in range(2):
        for c in range(4):
            for k in range(4):
                add16(f"wih{d}{c}{k}", wihT[d, c, k])
    for d in range(2):
        add16(f"whh{d}", whhT16[d])
    add16("h0", h0t)
    add16("id", ident16)
    add16("wor", worT16)
    add16("wj", wjT)
    p16_split = sum(b.shape[1] for b in p16_blocks)  # scan-critical prefix
    for c in range(8):
        for m in range(4):
            add16(f"wcm{c}{m}", wcmT[c, m])
    for c in range(8):
        for m in range(2):
            add16(f"wcv{c}{m}", wcvT[c, m])
    cpak16 = pack(p16_blocks)

    p32_blocks = []
    p32_off = {}

    def add32(name, arr):
        p32_off[name] = sum(b.shape[1] for b in p32_blocks)
        p32_blocks.append(arr.astype(np.float32))

    add32("bih", biht)
    add32("bcm", bcmt)
    add32("bcv", bcvt)
    add32("c0", c0t)
    cpak32 = pack(p32_blocks)

    in_maps = []
    for c in range(NC):
        xs = unit_emb[c * BC:(c + 1) * BC]                  # [8, 256, 512]
        xT0 = np.ascontiguousarray(xs.transpose(2, 0, 1)).reshape(4, H, BC * SEG)
        jm = np.concatenate(
            [jmasks[l][c * BC:(c + 1) * BC].reshape(1, BC * (SEGS[l] // 2))
             for l in range(7)], axis=1)                    # [1, 2032]
        jm = np.ascontiguousarray(np.broadcast_to(jm, (H, jm.shape[1])))
        in_maps.append(dict(
            xT0=xT0, cpak16=cpak16, cpak32=cpak32, jmask=jm,
        ))

    results = _runner(tuple(affines), tuple(sorted(p16_off.items())),
                  tuple(sorted(p32_off.items())), cpak16.shape[1],
                  cpak32.shape[1], p16_split)(in_maps)

    # ---- host assembly
    embeddings = np.empty((B, N_ORI, D), np.float32)
    embeddings[:, :SEG] = unit_emb
    rd = np.empty((B, N_ORI, 2), np.float32)
    joint_out = np.empty((B, N_JNT), np.float32)
    for c in range(NC):
        r = results[c]
        bs = slice(c * BC, (c + 1) * BC)
        for l in range(1, 8):
            nl = SEGS[l]
            xT = r[f"xT{l}"].reshape(D, BC, nl)
            embeddings[bs, OFF_ORI[l]:OFF_ORI[l] + nl] = xT.transpose(1, 2, 0)
        for l in range(8):
            nl = SEGS[l]
            off = int(OFF_ORI[l]) * BC
            f = r["rdf"][:, off:off + nl * BC].reshape(2, nl, BC)
            bk = r["rdb"][:, off:off + nl * BC].reshape(2, nl, BC)[:, ::-1]
            rd[bs, OFF_ORI[l]:OFF_ORI[l] + nl] = (f + bk).transpose(2, 1, 0)
        for l in range(7):
            nl = SEGS[l]
            off = int(OFF_JNT[l]) * BC
            blk = r["jout"][0, off:off + BC * (nl - 1)].reshape(BC, nl - 1)
            joint_out[bs, OFF_JNT[l]:OFF_JNT[l] + nl - 1] = blk
    rd += b_ori[None, None, :]
    joint_out += b_jnt[0]
    existence_out = np.ones((B, N_ORI), bool)
    return existence_out, embeddings, rd, joint_out


# ------------------------------------------------------------ device program
@functools.lru_cache(maxsize=2)
def _runner(affines, p16_items, p32_items, w16, w32, w16_split):
    import concourse.bass as bass
    import concourse.bacc as bacc
    import concourse.mybir as mybir
    import concourse.tile as tile
    from contextlib import ExitStack
    from concourse.bass_utils import run_bass_kernel_spmd

    f32, f16, f32r = mybir.dt.float32, mybir.dt.float16, mybir.dt.float32r
    AF = mybir.ActivationFunctionType
    ALU = mybir.AluOpType

    nc = bacc.Bacc("TRN2", target_bir_lowering=False, debug=False,
                   num_devices=NC)

    def din(name, shape, dt=f32):
        return nc.dram_tensor(name, list(shape), dt, kind="ExternalInput")

    def dext(name, shape, dt=f32):
        return nc.dram_tensor(name, list(shape), dt, kind="ExternalOutput")

    xT0 = din("xT0", [4, H, S0])
    cpak16 = din("cpak16", [H, w16], f16)
    cpak32 = din("cpak32", [H, w32])
    jmask = din("jmask", [H, BC * 254])
    o16 = dict(p16_items)
    o32 = dict(p32_items)

    xTs = {0: xT0}
    for l in range(1, 8):
        xTs[l] = dext(f"xT{l}", [4, H, BC * SEGS[l]])
    rdf = dext("rdf", [2, BC * N_ORI])
    rdb = dext("rdb", [2, BC * N_ORI])
    jout = dext("jout", [1, BC * N_JNT])

    with tile.TileContext(nc) as tc, ExitStack() as ctx:
        const_p = ctx.enter_context(tc.tile_pool(name="const", bufs=1))
        xstage = ctx.enter_context(tc.tile_pool(name="xstage", bufs=1))
        dense_ps = ctx.enter_context(tc.tile_pool(name="dps", bufs=1, space="PSUM"))
        dpj_ps = ctx.enter_context(tc.tile_pool(name="dpjp", bufs=1, space="PSUM"))
        dpr_ps = ctx.enter_context(tc.tile_pool(name="dprp", bufs=2, space="PSUM"))
        scan_ps = ctx.enter_context(tc.tile_pool(name="sps", bufs=2, space="PSUM"))
        work = ctx.enter_context(tc.tile_pool(name="work", bufs=1))
        scan_sb = ctx.enter_context(tc.tile_pool(name="scan", bufs=1))
        hs_p = ctx.enter_context(tc.tile_pool(name="hs", bufs=1))

        cp16 = const_p.tile([H, w16], f16, tag="cp16", name="cp16")
        nc.gpsimd.dma_start(cp16[:, :w16_split], cpak16[:, :w16_split])
        nc.gpsimd.dma_start(cp16[:, w16_split:], cpak16[:, w16_split:])
        cp32 = const_p.tile([H, w32], f32, tag="cp32", name="cp32")
        nc.gpsimd.dma_start(cp32[:], cpak32[:])
        jm_sb = const_p.tile([H, BC * 254], f32, tag="jm", name="jm_sb")
        nc.gpsimd.dma_start(jm_sb[:], jmask[:])

        def s16(name, w):
            return cp16[:, o16[name]:o16[name] + w]

        def s32(name, w):
            return cp32[:, o32[name]:o32[name] + w]

        wih_sb = [[[s16(f"wih{d}{c}{k}", H) for k in range(4)]
                   for c in range(4)] for d in range(2)]
        whh_sb = [s16(f"whh{d}", 512) for d in range(2)]
        bih_sb = s32("bih", 8)
        wcm_sb = [[s16(f"wcm{c}{m}", H) for m in range(4)] for c in range(8)]
        bcm_sb = s32("bcm", 4)
        wcv_sb = [[s16(f"wcv{c}{m}", H) for m in range(2)] for c in range(8)]
        bcv_sb = s32("bcv", 2)
        wj_sb = s16("wj", 2)
        wor_sb = s16("wor", 4)
        h0_sb = s16("h0", 2 * BC)
        c0_sb = s32("c0", 2 * BC)
        id_sb = s16("id", H)
        # dummy activation: absorb the one-time ACT table load early
        warm = work.tile([H, 2], f32, tag="warm", name="warm")
        nc.scalar.activation(warm[:, 0:1], cp32[:, 0:1], AF.Sigmoid)

        def absorb(tile_ap):
            # tiny DVE write to a just-stored tile: moves the DMA-WAR wait off
            # the next real writer (keeps every instruction <= 2 sync waits)
            nc.vector.tensor_copy(tile_ap, warm[0:1, 0:1])

        # persistent xp for layers >= 1: [128, (d, k, slot)] fp16
        xp_sb = const_p.tile([H, 2, 4, S0 + S1], f16)

        def ap3(t_ap, off, d1, d2):
            """free-dim AP [d1, d2] (outer, inner) at element offset off."""
            return bass.AP(t_ap.tensor, t_ap.offset + off,
                           [t_ap.ap[0], list(d1), list(d2)])

        # ================================================================
        def xp_pass(l, xt):
            """xp for layer l, both dirs, slot-major (t outer, b inner)."""
            nl = SEGS[l]
            R = BC * nl
            TCH = max(1, 512 // BC)  # tokens per chunk (64)
            for t0 in range(0, nl, TCH):
                for d in range(2):
                    tn = min(TCH, nl - t0)
                    nn = tn * BC
                    off = 0 if l == 0 else S0 + BC * int(sum(SEGS[1:l]))
                    for k in range(4):
                        ps = dense_ps.tile([H, 512], f32, tag="dp")
                        for c in range(4):
                            rhs = ap3(xt[c][:], t0, [1, tn], [nl, BC])
                            nc.tensor.matmul(ps[:, :nn],
                                             wih_sb[d][c][k],
                                             rhs,
                                             start=(c == 0), stop=(c == 3))
                        nc.scalar.activation(
                            xp_sb[:, d, k, off + t0 * BC: off + t0 * BC + nn],
                            ps[:, :nn], AF.Identity,
                            bias=bih_sb[:, 4 * d + k:4 * d + k + 1])

        # ================================================================
        def combine_pass(l, xt):
            nl = SEGS[l]
            n2 = nl // 2
            ls, lo, rs, ro = affines[l]
            jm_off = BC * int(OFF_X[l])
            NB = max(1, min(BC, 512 // max(n2, 1)))  # batch rows per tile
            for b0 in range(0, BC, NB):
                nb = min(NB, BC - b0)
                nn = nb * n2
                st = work.tile([H, 4, 512], f32, tag="cmb_st")

                def tok_ap(c, stride, off):
                    if stride == 0:
                        return ap3(xt[c][:], b0 * nl + off, [nl, nb], [0, n2])
                    return ap3(xt[c][:], b0 * nl + off, [nl, nb], [stride, n2])

                for m in range(4):
                    ps = dense_ps.tile([H, 512], f32, tag="dp")
                    for c in range(4):
                        nc.tensor.matmul(ps[:, :nn],
                                         wcm_sb[c][m],
                                         tok_ap(c, ls, lo),
                                         start=(c == 0), stop=False)
                    for c in range(4):
                        nc.tensor.matmul(ps[:, :nn],
                                         wcm_sb[4 + c][m],
                                         tok_ap(c, rs, ro),
                                         start=False, stop=(c == 3))
                    g = work.tile([H, 512], f32, tag="cmb_g")
                    nc.scalar.activation(g[:, :nn], ps[:, :nn], AF.Sigmoid,
                                         bias=bcm_sb[:, m:m + 1])
                    dd = work.tile([H, 512], f32, tag="cmb_d")
                    nc.vector.tensor_tensor(dd[:, :nn], tok_ap(m, ls, lo),
                                            tok_ap(m, rs, ro), op=ALU.subtract)
                    mm = work.tile([H, 512], f32, tag="cmb_m")
                    nc.vector.scalar_tensor_tensor(
                        mm[:, :nn], g[:, :nn], 1.0,
                        jm_sb[:, jm_off + b0 * n2: jm_off + b0 * n2 + nn],
                        op0=ALU.subtract, op1=ALU.mult)
                    nc.vector.tensor_tensor(mm[:, :nn], mm[:, :nn], dd[:, :nn],
                                            op=ALU.mult)
                    nc.vector.tensor_tensor(st[:, m, :nn], tok_ap(m, ls, lo),
                                            mm[:, :nn], op=ALU.add)
                nc.sync.dma_start(
                    xTs[l + 1].ap().rearrange("m h r -> h m r")[:, :, b0 * n2: b0 * n2 + nn],
                    st[:, :, :nn])
                absorb(st[0:1, 0, 0:1])

        # ================================================================
        def conv_pass(l, xt):
            nl = SEGS[l]
            nm1 = nl - 1
            joff = BC * int(OFF_JNT[l])
            NB = max(1, min(BC, 512 // nm1))
            for b0 in range(0, BC, NB):
                nb = min(NB, BC - b0)
                nn = nb * nm1
                rl = work.tile([H, 2, 512], f16, tag="cv_r")
                for m in range(2):
                    ps = dense_ps.tile([H, 512], f32, tag="dp")
                    for c in range(4):
                        rhs = ap3(xt[c][:], b0 * nl, [nl, nb], [1, nm1])
                        nc.tensor.matmul(ps[:, :nn],
                                         wcv_sb[c][m],
                                         rhs,
                                         start=(c == 0), stop=False)
                    for c in range(4):
                        rhs = ap3(xt[c][:], b0 * nl + 1, [nl, nb], [1, nm1])
                        nc.tensor.matmul(ps[:, :nn],
                                         wcv_sb[4 + c][m],
                                         rhs,
                                         start=False, stop=(c == 3))
                    nc.scalar.activation(rl[:, m, :nn], ps[:, :nn], AF.Relu,
                                         bias=bcv_sb[:, m:m + 1])
                pj = dpj_ps.tile([1, 512], f32, tag="dpj", name=f"pj{l}_{b0}")
                for m in range(2):
                    nc.tensor.matmul(pj[:1, :nn], wj_sb[:, m:m + 1],
                                     rl[:, m, :nn],
                                     start=(m == 0), stop=(m == 1))
                jsb = work.tile([1, 512], f32, tag="cv_j")
                nc.vector.tensor_copy(jsb[:1, :nn], pj[:1, :nn])
                nc.sync.dma_start(
                    jout[0:1, joff + b0 * nm1: joff + b0 * nm1 + nn],
                    jsb[:1, :nn])
                absorb(jsb[0:1, 0:1])

        # ================================================================
        def scan_pass(l):
            nl = SEGS[l]
            g = "A" if l == 0 else "B"  # separate tag group per chain so the
            # layer-0 chain and the layers-1..7 chain run concurrently
            hs = hs_p.tile([H, nl, 2, BC], f16, tag=f"hs{l}", name=f"hs{l}")
            ct = scan_sb.tile([H, 2, BC], f32, tag=f"ct{g}", name=f"ct{l}")
            nc.vector.tensor_copy(ct[:], c0_sb)
            xoff = 0 if l == 0 else S0 + BC * int(sum(SEGS[1:l]))
            for s in range(nl):
                ps = scan_ps.tile([H, 4, 2, BC], f32, tag=f"g{g}",
                                  name=f"ps{l}_{s}")
                # xp inject (identity matmul), per dir
                for d in range(2):
                    t = s if d == 0 else nl - 1 - s
                    rhs = xp_sb[:, d, :, xoff + t * BC: xoff + (t + 1) * BC]
                    nc.tensor.matmul(ps[:, :, d, :], id_sb, rhs,
                                     start=(d == 0), stop=False)
                # whh gate matmuls
                for d in range(2):
                    hprev = (h0_sb[:, d * BC:(d + 1) * BC] if s == 0
                             else hs[:, s - 1, d, :])
                    for k in range(4):
                        nc.tensor.matmul(ps[:, k, d, :],
                                         whh_sb[d][:, k * H:(k + 1) * H],
                                         hprev, start=False,
                                         stop=(d == 1 and k == 3))
                sg = scan_sb.tile([H, 4, 2, BC], f32, tag=f"sg{g}",
                                  name=f"sg{l}_{s}")
                nc.scalar.activation(sg[:], ps[:], AF.Sigmoid)
                # c' = sig(f)*c + sig(i)*(2*sig(2g) - 1), via
                # w = (sig(2g) - 0.5)*sig(i); c' = 2*w + sig(f)*c
                w = scan_sb.tile([H, 2, BC], f32, tag=f"w{g}",
                                 name=f"w{l}_{s}")
                nc.vector.scalar_tensor_tensor(
                    w[:], sg[:, 3, :, :], 0.5, sg[:, 1, :, :],
                    op0=ALU.subtract, op1=ALU.mult)
                vlo = scan_sb.tile([H, 2, BC], f32, tag=f"v{g}",
                                   name=f"v{l}_{s}")
                nc.vector.tensor_tensor(vlo[:], sg[:, 0, :, :], ct[:],
                                        op=ALU.mult)
                nc.vector.scalar_tensor_tensor(
                    ct[:], w[:], 2.0, vlo[:], op0=ALU.mult, op1=ALU.add)
                th = scan_sb.tile([H, 2, BC], f32, tag=f"th{g}",
                                  name=f"th{l}_{s}")
                nc.scalar.activation(th[:], ct[:], AF.Tanh)
                nc.vector.tensor_tensor(hs[:, s, :, :], sg[:, 2, :, :], th[:],
                                        op=ALU.mult)
            # rd projection for this layer, slot-major output
            roff = BC * int(OFF_ORI[l])
            CH = 512 // BC  # slots per chunk
            for d, out in ((0, rdf), (1, rdb)):
                for s0 in range(0, nl, CH):
                    sn = min(CH, nl - s0)
                    nn = sn * BC
                    pr = dpr_ps.tile([2, 512], f32, tag="dpr", name=f"pr{l}_{d}_{s0}")
                    nc.tensor.matmul(pr[:2, :nn], wor_sb[:, 2 * d:2 * d + 2],
                                     hs[:, s0:s0 + sn, d, :], start=True,
                                     stop=True)
                    rsb = work.tile([2, 512], f32, tag=f"rsb{g}",
                                    name=f"rsb{l}_{d}_{s0}")
                    nc.vector.tensor_copy(rsb[:2, :nn], pr[:2, :nn])
                    nc.sync.dma_start(
                        out[0:2, roff + s0 * BC: roff + s0 * BC + nn],
                        rsb[:2, :nn])
                    absorb(rsb[0:1, 0:1])

        # ================================================================
        for l in range(8):
            nl = SEGS[l]
            xt = [xstage.tile([H, BC * nl], f16, tag=f"xt{c}", name=f"xt{c}_{l}")
                  for c in range(4)]
            for c in range(4):
                nc.gpsimd.dma_start(xt[c][:], xTs[l][c])
            xp_pass(l, xt)
            if l < 7:
                combine_pass(l, xt)
                conv_pass(l, xt)
            scan_pass(l)

    nc.finalize()

    def run(in_maps):
        import time
        import kernel as _self
        t0 = time.time()
        res = run_bass_kernel_spmd(nc, in_maps, list(range(NC)))
        _self.LAST_EXEC_NS = int((time.time() - t0) * 1e9)
        if res.exec_time_ns:
            _self.LAST_EXEC_NS = int(res.exec_time_ns)
        return res.results

    return run


# revision 29
# speedup vs baseline: 1.1084x; 1.1084x over previous
"""Trainium2 Bass kernel for nn_DiscoStem (B=64, 8 layers of halving BiLSTM).

Sharding: pure data parallel over batch (B=64 -> 8 cores x 8 rows each), per
the sharding hint; all parameters replicated.  Each core runs the whole layer
pipeline for its batch shard in ONE SPMD Bass program:

  - x is kept transposed per layer in DRAM: xT_l[4][128][BC*n] (d-major, token
    column = b*n + t), so every matmul contraction is a plain [128, N] tile
    and all token gathers / conv shifts are free-dim strided APs.
  - LSTM scan (the latency-critical part): gate tiles [128=H, (gate,dir,b)],
    fwd+bwd chains share each step's instructions.  Per step: 2 xp-inject
    matmuls (identity trick, fp16) + 8 whh gate matmuls (fp16, FWL) + one
    fused sigmoid over (f,i,o), tanh(g), then 3 packed DVE ops for the
    c/h update.  xp is precomputed slot-major per layer (one big fp32r GEMM).
  - split/merge gathers: host computes the exact reference indices; affine
    patterns (all reference-style inputs: clean pairing, or all-ones condense)
    are baked as strided APs.  Non-affine patterns are not supported.
  - combine gate + blend: sigmoid on ACT; x' = lhs + (g-1)*jnt*(lhs-rhs) via
    scalar_tensor_tensor + tensor_tensor ops.

Host side: prepares transposed/permuted weights (gate order f,i,o,g), runs the
SPMD program on 8 cores via run_bass_kernel_spmd, then assembles the outputs
(rd = rd_f + rd_b + b_ori with bwd slots reversed; embeddings un-transposed;
existence = ones).
"""

import functools

import numpy as np

# ---------------------------------------------------------------- constants
B, SEG, D, OD, H = 64, 256, 512, 256, 128
SEGS = [256, 128, 64, 32, 16, 8, 4, 2]
N_ORI = sum(SEGS)                      # 510
N_JNT = sum(s - 1 for s in SEGS[:-1])  # 501
NC = 8                                 # cores
BC = B // NC                           # batch rows per core (8)
GSRC = [1, 0, 3, 2]                    # target gate k (f,i,o,g) -> pytorch block

OFF_ORI = np.concatenate([[0], np.cumsum(SEGS)]).astype(np.int64)
OFF_JNT = np.concatenate([[0], np.cumsum([s - 1 for s in SEGS[:-1]])]).astype(np.int64)
OFF_X = np.concatenate([[0], np.cumsum([s // 2 for s in SEGS[:-1]])]).astype(np.int64)
S0 = BC * SEG                          # layer-0 scan slots per core
S1 = BC * sum(SEGS[1:])                # layer>=1 scan slots per core (2032)


# ------------------------------------------------------------- host helpers
def _split_merge_idx(right, joint, exist):
    """Exact numpy replica of reference._split_merge index math."""
    bsz, n = right.shape
    n2 = n // 2
    agree = right[:, :-1] & ~right[:, 1:]
    pj = agree & joint
    rhs_is = np.concatenate([np.zeros((bsz, 1), bool), pj], axis=1)
    node_start = exist & ~rhs_is
    node_id = np.cumsum(node_start.astype(np.int64), axis=1) - 1
    pos = np.broadcast_to(np.arange(n, dtype=np.int64), (bsz, n))
    lhs_idx = np.zeros((bsz, n2), np.int64)
    rhs_idx = np.zeros((bsz, n2), np.int64)
    jnt = np.zeros((bsz, n2), bool)
    tgt_l = np.where(node_start, node_id, n2)
    lhs_pj = np.concatenate([pj, np.zeros((bsz, 1), bool)], axis=1)
    tgt_r = np.where(lhs_pj, node_id, n2)
    for b in range(bsz):
        ok = tgt_l[b] < n2
        lhs_idx[b, tgt_l[b][ok]] = pos[b, ok]
        ok = tgt_r[b] < n2
        rhs_idx[b, tgt_r[b][ok]] = pos[b, ok] + 1
        jnt[b, tgt_r[b][ok]] = True
    return lhs_idx, rhs_idx, jnt


def _affine_of(idx):
    """If idx[b, i] == s*i + o for all b, return (s, o); else None."""
    bsz, n2 = idx.shape
    if n2 == 1:
        return (1, int(idx[0, 0])) if (idx == idx[0, 0]).all() else None
    s = int(idx[0, 1] - idx[0, 0])
    o = int(idx[0, 0])
    if s < 0:
        return None
    ref = s * np.arange(n2, dtype=np.int64)[None, :] + o
    return (s, o) if (idx == ref).all() else None


# --------------------------------------------------------------- the kernel
def kernel(**inputs):
    unit_emb = np.asarray(inputs["unit_emb"], np.float32)
    existence = np.asarray(inputs["existence"], bool)
    sup_right = np.asarray(inputs["supervised_right"], bool)
    sup_joint = np.asarray(inputs["supervised_joint"], bool)
    h0 = np.asarray(inputs["h0"], np.float32)
    c0 = np.asarray(inputs["c0"], np.float32)
    b_ori = np.asarray(inputs["b_ori"], np.float32)
    b_jnt = np.asarray(inputs["b_jnt"], np.float32)

    # ---- host: per-layer gather indices (exact) + affine detection
    ex = existence
    ori_off = jnt_off = 0
    affines = []
    jmasks = []
    n = SEG
    for l in range(7):
        right = sup_right[:, ori_off:ori_off + n]; ori_off += n
        joint = sup_joint[:, jnt_off:jnt_off + n - 1]; jnt_off += n - 1
        lhs_idx, rhs_idx, jnt = _split_merge_idx(right, joint, ex)
        la, ra = _affine_of(lhs_idx), _affine_of(rhs_idx)
        if la is None or ra is None:
            raise NotImplementedError(
                "non-affine split/merge index pattern is not supported")
        affines.append((*la, *ra))
        jmasks.append(jnt.astype(np.float32))
        ex = np.ones((B, n // 2), bool)
        n //= 2

    # ---- host: weights in device layouts (gate order f,i,o,g)
    def perm_gates(w):
        return np.concatenate([w[g * H:(g + 1) * H] for g in GSRC], axis=0)

    wih = {0: perm_gates(np.asarray(inputs["wih_f"], np.float32)),
           1: perm_gates(np.asarray(inputs["wih_b"], np.float32))}
    whh = {0: perm_gates(np.asarray(inputs["whh_f"], np.float32)),
           1: perm_gates(np.asarray(inputs["whh_b"], np.float32))}
    bih = {0: perm_gates(np.asarray(inputs["b_f"], np.float32)[:, None])[:, 0],
           1: perm_gates(np.asarray(inputs["b_b"], np.float32)[:, None])[:, 0]}
    # tanh(g) = 2*sigmoid(2g) - 1: fold the 2x into the g-gate block so the
    # scan needs a single fused sigmoid over all four gates
    for d in (0, 1):
        wih[d] = wih[d].copy(); whh[d] = whh[d].copy(); bih[d] = bih[d].copy()
        wih[d][3 * H:4 * H] *= 2.0
        whh[d][3 * H:4 * H] *= 2.0
        bih[d][3 * H:4 * H] *= 2.0
    w_ori = np.asarray(inputs["w_ori"], np.float32)
    w_cnv = np.asarray(inputs["w_cnv"], np.float32)
    b_cnv = np.asarray(inputs["b_cnv"], np.float32)
    w_jnt = np.asarray(inputs["w_jnt"], np.float32)
    w_cmb = np.asarray(inputs["w_cmb"], np.float32)
    b_cmb = np.asarray(inputs["b_cmb"], np.float32)

    wihT = np.stack([
        np.stack([np.stack([wih[d][k * H:(k + 1) * H, c * H:(c + 1) * H].T
                            for k in range(4)]) for c in range(4)])
        for d in (0, 1)]).astype(np.float16)                # [2,4,4,128,128]
    whhT16 = np.stack([whh[d].T for d in (0, 1)]).astype(np.float16)  # [2,128,512]
    biht = np.stack([bih[d].reshape(4, H).T for d in (0, 1)], axis=0)
    biht = np.ascontiguousarray(biht.transpose(1, 0, 2)).reshape(H, 8)  # [128,(d,k)]
    wcmT = np.stack([np.stack([w_cmb[c * H:(c + 1) * H, m * H:(m + 1) * H]
                               for m in range(4)]) for c in range(8)]).astype(np.float16)
    bcmt = np.ascontiguousarray(b_cmb.reshape(4, H).T)      # [128, 4]
    wcv = np.concatenate([w_cnv[0], w_cnv[1]], axis=0)      # [1024, 256]
    wcvT = np.stack([np.stack([wcv[c * H:(c + 1) * H, m * H:(m + 1) * H]
                               for m in range(2)]) for c in range(8)]).astype(np.float16)
    bcvt = np.ascontiguousarray(b_cnv.reshape(2, H).T)      # [128, 2]
    wjT = np.ascontiguousarray(w_jnt[:, 0].reshape(2, H).T).astype(np.float16)
    worT16 = np.concatenate([w_ori[:H], w_ori[H:]], axis=1).astype(np.float16)
    h0i = np.tanh(h0)
    h0t = np.concatenate([np.broadcast_to(h0i[d, 0][:, None], (H, BC))
                          for d in (0, 1)], axis=1).astype(np.float16)
    c0t = np.concatenate([np.broadcast_to(c0[d, 0][:, None], (H, BC))
                          for d in (0, 1)], axis=1).astype(np.float32)
    ident16 = np.eye(H, dtype=np.float16)

    # pack all constant tensors into two [H, X] arrays (one DMA each)
    def pack(blocks):
        cols = np.concatenate(blocks, axis=1)
        return np.ascontiguousarray(cols)

    p16_blocks = []
    p16_off = {}

    def add16(name, arr):
        p16_off[name] = sum(b.shape[1] for b in p16_blocks)
        p16_blocks.append(arr.astype(np.float16))

    for d in range(2):
        for c in range(4):
            for k in range(4):
                add16(f"wih{d}{c}{k}", wihT[d, c, k])
    for d in range(2):
        add16(f"whh{d}", whhT16[d])
    add16("h0", h0t)
    add16("id", ident16)
    add16("wor", worT16)
    add16("wj", wjT)
    p16_split = sum(b.shape[1] for b in p16_blocks)  # scan-critical prefix
    for c in range(8):
        for m in range(4):
            add16(f"wcm{c}{m}", wcmT[c, m])
    for c in range(8):
        for m in range(2):
            add16(f"wcv{c}{m}", wcvT[c, m])
    cpak16 = pack(p16_blocks)

    p32_blocks = []
    p32_off = {}

    def add32(name, arr):
        p32_off[name] = sum(b.shape[1] for b in p32_blocks)
        p32_blocks.append(arr.astype(np.float32))

    add32("bih", biht)
    add32("bcm", bcmt)
    add32("bcv", bcvt)
    add32("c0", c0t)
    cpak32 = pack(p32_blocks)

    in_maps = []
    for c in range(NC):
        xs = unit_emb[c * BC:(c + 1) * BC]                  # [8, 256, 512]
        xT0 = np.ascontiguousarray(xs.transpose(2, 0, 1)).reshape(4, H, BC * SEG)
        jm = np.concatenate(
            [jmasks[l][c * BC:(c + 1) * BC].reshape(1, BC * (SEGS[l] // 2))
             for l in range(7)], axis=1)                    # [1, 2032]
        jm = np.ascontiguousarray(np.broadcast_to(jm, (H, jm.shape[1])))
        in_maps.append(dict(
            xT0=xT0, cpak16=cpak16, cpak32=cpak32, jmask=jm,
        ))

    results = _runner(tuple(affines), tuple(sorted(p16_off.items())),
                  tuple(sorted(p32_off.items())), cpak16.shape[1],
                  cpak32.shape[1], p16_split)(in_maps)

    # ---- host assembly
    embeddings = np.empty((B, N_ORI, D), np.float32)
    embeddings[:, :SEG] = unit_emb
    rd = np.empty((B, N_ORI, 2), np.float32)
    joint_out = np.empty((B, N_JNT), np.float32)
    for c in range(NC):
        r = results[c]
        bs = slice(c * BC, (c + 1) * BC)
        for l in range(1, 8):
            nl = SEGS[l]
            xT = r[f"xT{l}"].reshape(D, BC, nl)
            embeddings[bs, OFF_ORI[l]:OFF_ORI[l] + nl] = xT.transpose(1, 2, 0)
        for l in range(8):
            nl = SEGS[l]
            off = int(OFF_ORI[l]) * BC
            f = r["rdf"][:, off:off + nl * BC].reshape(2, nl, BC)
            bk = r["rdb"][:, off:off + nl * BC].reshape(2, nl, BC)[:, ::-1]
            rd[bs, OFF_ORI[l]:OFF_ORI[l] + nl] = (f + bk).transpose(2, 1, 0)
        for l in range(7):
            nl = SEGS[l]
            off = int(OFF_JNT[l]) * BC
            blk = r["jout"][0, off:off + BC * (nl - 1)].reshape(BC, nl - 1)
            joint_out[bs, OFF_JNT[l]:OFF_JNT[l] + nl - 1] = blk
    rd += b_ori[None, None, :]
    joint_out += b_jnt[0]
    existence_out = np.ones((B, N_ORI), bool)
    return existence_out, embeddings, rd, joint_out


# ------------------------------------------------------------ device program
@functools.lru_cache(maxsize=2)
def _runner(affines, p16_items, p32_items, w16, w32, w16_split):
    import concourse.bass as bass
    import concourse.bacc as bacc
    import concourse.mybir as mybir
    import concourse.tile as tile
    from contextlib import ExitStack
    from concourse.bass_utils import run_bass_kernel_spmd

    f32, f16, f32r = mybir.dt.float32, mybir.dt.float16, mybir.dt.float32r
    AF = mybir.ActivationFunctionType
    ALU = mybir.AluOpType

    nc = bacc.Bacc("TRN2", target_bir_lowering=False, debug=False,
                   num_devices=NC)

    def din(name, shape, dt=f32):
        return nc.dram_tensor(name, list(shape), dt, kind="ExternalInput")

    def dext(name, shape, dt=f32):
        return nc.dram_tensor(name, list(shape), dt, kind="ExternalOutput")

    xT0 = din("xT0", [4, H, S0])
    cpak16 = din("cpak16", [H, w16], f16)
    cpak32 = din("cpak32", [H, w32])
    jmask = din("jmask", [H, BC * 254])
    o16 = dict(p16_items)
    o32 = dict(p32_items)

    xTs = {0: xT0}
    for l in range(1, 8):
        xTs[l] = dext(f"xT{l}", [4, H, BC * SEGS[l]])
    rdf = dext("rdf", [2, BC * N_ORI])
    rdb = dext("rdb", [2, BC * N_ORI])
    jout = dext("jout", [1, BC * N_JNT])

    with tile.TileContext(nc) as tc, ExitStack() as ctx:
        const_p = ctx.enter_context(tc.tile_pool(name="const", bufs=1))
        xstage = ctx.enter_context(tc.tile_pool(name="xstage", bufs=1))
        dense_ps = ctx.enter_context(tc.tile_pool(name="dps", bufs=1, space="PSUM"))
        dpj_ps = ctx.enter_context(tc.tile_pool(name="dpjp", bufs=1, space="PSUM"))
        dpr_ps = ctx.enter_context(tc.tile_pool(name="dprp", bufs=2, space="PSUM"))
        scan_psA = ctx.enter_context(tc.tile_pool(name="spsA", bufs=2, space="PSUM"))
        scan_psB1 = ctx.enter_context(tc.tile_pool(name="spsB1", bufs=1, space="PSUM"))
        scan_psBr = ctx.enter_context(tc.tile_pool(name="spsBr", bufs=1, space="PSUM"))
        work = ctx.enter_context(tc.tile_pool(name="work", bufs=1))
        scan_sb = ctx.enter_context(tc.tile_pool(name="scan", bufs=1))
        hs_p = ctx.enter_context(tc.tile_pool(name="hs", bufs=1))

        cp16 = const_p.tile([H, w16], f16, tag="cp16", name="cp16")
        nc.gpsimd.dma_start(cp16[:, :w16_split], cpak16[:, :w16_split])
        cp32 = const_p.tile([H, w32], f32, tag="cp32", name="cp32")
        nc.gpsimd.dma_start(cp32[:], cpak32[:])
        # layer-0 x staging jumps the SWDGE queue ahead of the non-critical
        # constants (wcm/wcv, join masks) so the L0 scan can start sooner
        xt0_tiles = [xstage.tile([H, BC * SEG], f16, tag=f"xt{c}",
                                 name=f"xt{c}_0") for c in range(4)]
        for c in range(4):
            nc.gpsimd.dma_start(xt0_tiles[c][:], xTs[0][c])
        nc.gpsimd.dma_start(cp16[:, w16_split:], cpak16[:, w16_split:])
        jm_sb = const_p.tile([H, BC * 254], f32, tag="jm", name="jm_sb")
        nc.gpsimd.dma_start(jm_sb[:], jmask[:])

        def s16(name, w):
            return cp16[:, o16[name]:o16[name] + w]

        def s32(name, w):
            return cp32[:, o32[name]:o32[name] + w]

        wih_sb = [[[s16(f"wih{d}{c}{k}", H) for k in range(4)]
                   for c in range(4)] for d in range(2)]
        whh_sb = [s16(f"whh{d}", 512) for d in range(2)]
        bih_sb = s32("bih", 8)
        wcm_sb = [[s16(f"wcm{c}{m}", H) for m in range(4)] for c in range(8)]
        bcm_sb = s32("bcm", 4)
        wcv_sb = [[s16(f"wcv{c}{m}", H) for m in range(2)] for c in range(8)]
        bcv_sb = s32("bcv", 2)
        wj_sb = s16("wj", 2)
        wor_sb = s16("wor", 4)
        h0_sb = s16("h0", 2 * BC)
        c0_sb = s32("c0", 2 * BC)
        id_sb = s16("id", H)
        # dummy activation: absorb the one-time ACT table load early
        warm = work.tile([H, 2], f32, tag="warm", name="warm")
        nc.scalar.activation(warm[:, 0:1], cp32[:, 0:1], AF.Sigmoid)

        def absorb(tile_ap):
            # tiny DVE write to a just-stored tile: moves the DMA-WAR wait off
            # the next real writer (keeps every instruction <= 2 sync waits)
            nc.vector.tensor_copy(tile_ap, warm[0:1, 0:1])

        # persistent xp for layers >= 1: [128, (d, k, slot)] fp16
        xp_sb = const_p.tile([H, 2, 4, S0 + S1], f16)

        def ap3(t_ap, off, d1, d2):
            """free-dim AP [d1, d2] (outer, inner) at element offset off."""
            return bass.AP(t_ap.tensor, t_ap.offset + off,
                           [t_ap.ap[0], list(d1), list(d2)])

        # ================================================================
        def xp_pass(l, xt):
            """xp for layer l, both dirs, slot-major (t outer, b inner)."""
            nl = SEGS[l]
            R = BC * nl
            TCH = max(1, 512 // BC)  # tokens per chunk (64)
            for t0 in range(0, nl, TCH):
                for d in range(2):
                    tn = min(TCH, nl - t0)
                    nn = tn * BC
                    off = 0 if l == 0 else S0 + BC * int(sum(SEGS[1:l]))
                    for k in range(4):
                        ps = dense_ps.tile([H, 512], f32, tag="dp")
                        for c in range(4):
                            rhs = ap3(xt[c][:], t0, [1, tn], [nl, BC])
                            nc.tensor.matmul(ps[:, :nn],
                                             wih_sb[d][c][k],
                                             rhs,
                                             start=(c == 0), stop=(c == 3))
                        nc.scalar.activation(
                            xp_sb[:, d, k, off + t0 * BC: off + t0 * BC + nn],
                            ps[:, :nn], AF.Identity,
                            bias=bih_sb[:, 4 * d + k:4 * d + k + 1])

        # ================================================================
        def combine_pass(l, xt):
            nl = SEGS[l]
            n2 = nl // 2
            ls, lo, rs, ro = affines[l]
            jm_off = BC * int(OFF_X[l])
            NB = max(1, min(BC, 512 // max(n2, 1)))  # batch rows per tile
            for b0 in range(0, BC, NB):
                nb = min(NB, BC - b0)
                nn = nb * n2
                st = work.tile([H, 4, 512], f32, tag="cmb_st")

                def tok_ap(c, stride, off):
                    if stride == 0:
                        return ap3(xt[c][:], b0 * nl + off, [nl, nb], [0, n2])
                    return ap3(xt[c][:], b0 * nl + off, [nl, nb], [stride, n2])

                for m in range(4):
                    ps = dense_ps.tile([H, 512], f32, tag="dp")
                    for c in range(4):
                        nc.tensor.matmul(ps[:, :nn],
                                         wcm_sb[c][m],
                                         tok_ap(c, ls, lo),
                                         start=(c == 0), stop=False)
                    for c in range(4):
                        nc.tensor.matmul(ps[:, :nn],
                                         wcm_sb[4 + c][m],
                                         tok_ap(c, rs, ro),
                                         start=False, stop=(c == 3))
                    g = work.tile([H, 512], f32, tag="cmb_g")
                    nc.scalar.activation(g[:, :nn], ps[:, :nn], AF.Sigmoid,
                                         bias=bcm_sb[:, m:m + 1])
                    dd = work.tile([H, 512], f32, tag="cmb_d")
                    nc.vector.tensor_tensor(dd[:, :nn], tok_ap(m, ls, lo),
                                            tok_ap(m, rs, ro), op=ALU.subtract)
                    mm = work.tile([H, 512], f32, tag="cmb_m")
                    nc.vector.scalar_tensor_tensor(
                        mm[:, :nn], g[:, :nn], 1.0,
                        jm_sb[:, jm_off + b0 * n2: jm_off + b0 * n2 + nn],
                        op0=ALU.subtract, op1=ALU.mult)
                    nc.vector.tensor_tensor(mm[:, :nn], mm[:, :nn], dd[:, :nn],
                                            op=ALU.mult)
                    nc.vector.tensor_tensor(st[:, m, :nn], tok_ap(m, ls, lo),
                                            mm[:, :nn], op=ALU.add)
                nc.sync.dma_start(
                    xTs[l + 1].ap().rearrange("m h r -> h m r")[:, :, b0 * n2: b0 * n2 + nn],
                    st[:, :, :nn])
                absorb(st[0:1, 0, 0:1])

        # ================================================================
        def conv_pass(l, xt):
            nl = SEGS[l]
            nm1 = nl - 1
            joff = BC * int(OFF_JNT[l])
            NB = max(1, min(BC, 512 // nm1))
            for b0 in range(0, BC, NB):
                nb = min(NB, BC - b0)
                nn = nb * nm1
                rl = work.tile([H, 2, 512], f16, tag="cv_r")
                for m in range(2):
                    ps = dense_ps.tile([H, 512], f32, tag="dp")
                    for c in range(4):
                        rhs = ap3(xt[c][:], b0 * nl, [nl, nb], [1, nm1])
                        nc.tensor.matmul(ps[:, :nn],
                                         wcv_sb[c][m],
                                         rhs,
                                         start=(c == 0), stop=False)
                    for c in range(4):
                        rhs = ap3(xt[c][:], b0 * nl + 1, [nl, nb], [1, nm1])
                        nc.tensor.matmul(ps[:, :nn],
                                         wcv_sb[4 + c][m],
                                         rhs,
                                         start=False, stop=(c == 3))
                    nc.scalar.activation(rl[:, m, :nn], ps[:, :nn], AF.Relu,
                                         bias=bcv_sb[:, m:m + 1])
                pj = dpj_ps.tile([1, 512], f32, tag="dpj", name=f"pj{l}_{b0}")
                for m in range(2):
                    nc.tensor.matmul(pj[:1, :nn], wj_sb[:, m:m + 1],
                                     rl[:, m, :nn],
                                     start=(m == 0), stop=(m == 1))
                jsb = work.tile([1, 512], f32, tag="cv_j")
                nc.vector.tensor_copy(jsb[:1, :nn], pj[:1, :nn])
                nc.sync.dma_start(
                    jout[0:1, joff + b0 * nm1: joff + b0 * nm1 + nn],
                    jsb[:1, :nn])
                absorb(jsb[0:1, 0:1])

        # ================================================================
        def scan_pass(l):
            nl = SEGS[l]
            # per-layer tags: all 8 layer scans are independent chains and
            # can run concurrently (L2-7 share one psum tag, staying serial
            # among themselves, which is fine - they total 126 steps)
            g = str(l)
            psp = scan_psA if l == 0 else (scan_psB1 if l == 1 else scan_psBr)
            pst = "gA" if l == 0 else ("gB1" if l == 1 else "gBr")
            hs = hs_p.tile([H, nl, 2, BC], f16, tag=f"hs{l}", name=f"hs{l}")
            ct = scan_sb.tile([H, 2, BC], f32, tag=f"ct{g}", name=f"ct{l}")
            nc.vector.tensor_copy(ct[:], c0_sb)
            xoff = 0 if l == 0 else S0 + BC * int(sum(SEGS[1:l]))
            for s in range(nl):
                ps = psp.tile([H, 4, 2, BC], f32, tag=pst,
                              name=f"ps{l}_{s}")
                # xp inject (identity matmul), per dir
                for d in range(2):
                    t = s if d == 0 else nl - 1 - s
                    rhs = xp_sb[:, d, :, xoff + t * BC: xoff + (t + 1) * BC]
                    nc.tensor.matmul(ps[:, :, d, :], id_sb, rhs,
                                     start=(d == 0), stop=False)
                # whh gate matmuls
                for d in range(2):
                    hprev = (h0_sb[:, d * BC:(d + 1) * BC] if s == 0
                             else hs[:, s - 1, d, :])
                    for k in range(4):
                        nc.tensor.matmul(ps[:, k, d, :],
                                         whh_sb[d][:, k * H:(k + 1) * H],
                                         hprev, start=False,
                                         stop=(d == 1 and k == 3))
                sg = scan_sb.tile([H, 4, 2, BC], f32, tag=f"sg{g}",
                                  name=f"sg{l}_{s}")
                nc.scalar.activation(sg[:], ps[:], AF.Sigmoid)
                # c' = sig(f)*c + sig(i)*(2*sig(2g) - 1), via
                # w = (sig(2g) - 0.5)*sig(i); c' = 2*w + sig(f)*c
                w = scan_sb.tile([H, 2, BC], f32, tag=f"w{g}",
                                 name=f"w{l}_{s}")
                nc.vector.scalar_tensor_tensor(
                    w[:], sg[:, 3, :, :], 0.5, sg[:, 1, :, :],
                    op0=ALU.subtract, op1=ALU.mult)
                vlo = scan_sb.tile([H, 2, BC], f32, tag=f"v{g}",
                                   name=f"v{l}_{s}")
                nc.vector.tensor_tensor(vlo[:], sg[:, 0, :, :], ct[:],
                                        op=ALU.mult)
                nc.vector.scalar_tensor_tensor(
                    ct[:], w[:], 2.0, vlo[:], op0=ALU.mult, op1=ALU.add)
                th = scan_sb.tile([H, 2, BC], f32, tag=f"th{g}",
                                  name=f"th{l}_{s}")
                nc.scalar.activation(th[:], ct[:], AF.Tanh)
                nc.vector.tensor_tensor(hs[:, s, :, :], sg[:, 2, :, :], th[:],
                                        op=ALU.mult)
            # rd projection for this layer, slot-major output
            roff = BC * int(OFF_ORI[l])
            CH = 512 // BC  # slots per chunk
            for d, out in ((0, rdf), (1, rdb)):
                for s0 in range(0, nl, CH):
                    sn = min(CH, nl - s0)
                    nn = sn * BC
                    pr = dpr_ps.tile([2, 512], f32, tag="dpr", name=f"pr{l}_{d}_{s0}")
                    nc.tensor.matmul(pr[:2, :nn], wor_sb[:, 2 * d:2 * d + 2],
                                     hs[:, s0:s0 + sn, d, :], start=True,
                                     stop=True)
                    rsb = work.tile([2, 512], f32, tag=f"rsb{g}",
                                    name=f"rsb{l}_{d}_{s0}")
                    nc.vector.tensor_copy(rsb[:2, :nn], pr[:2, :nn])
                    nc.sync.dma_start(
                        out[0:2, roff + s0 * BC: roff + s0 * BC + nn],
                        rsb[:2, :nn])
                    absorb(rsb[0:1, 0:1])

        # ================================================================
        for l in range(8):
            nl = SEGS[l]
            if l == 0:
                xt = xt0_tiles
            else:
                xt = [xstage.tile([H, BC * nl], f16, tag=f"xt{c}",
                                  name=f"xt{c}_{l}") for c in range(4)]
                for c in range(4):
                    nc.gpsimd.dma_start(xt[c][:], xTs[l][c])
            xp_pass(l, xt)
            # scan first: its instructions outrank the dense tail in scheduler
            # priority, keeping the critical L0 chain unimpeded
            scan_pass(l)
            if l < 7:
                combine_pass(l, xt)
                conv_pass(l, xt)

    nc.finalize()

    def run(in_maps):
        import time
        import kernel as _self
        t0 = time.time()
        res = run_bass_kernel_spmd(nc, in_maps, list(range(NC)))
        _self.LAST_EXEC_NS = int((time.time() - t0) * 1e9)
        if res.exec_time_ns:
            _self.LAST_EXEC_NS = int(res.exec_time_ns)
        return res.results

    return run
